# revision 82
# baseline (speedup 1.0000x reference)
"""Trainium2 Bass kernel for nn_LocalState_1580547972191 (sparse_attention).

Contract: kernel(**inputs) takes FULL unsharded inputs (as from setup_inputs()),
returns FULL output [4, 512, 2048] f32. Internally shards across 8 NeuronCores:
core = (b, hg) with b = batch, hg = head-group (heads {2hg, 2hg+1}).

Algorithm (per core), validated against the reference in fp64/fp32:
- The decay bias -g(s)|t-s| with g(s) >= ~0.28 makes attention effectively
  banded: each 128-query block attends to a 384-wide key window (128-aligned).
- Freq bias cos(2*pi*(t-s)/p) = cos_p(t)cos_p(s) + sin_p(t)sin_p(s) is rank-2:
  folded into the QK^T matmul via 8 augmented rows.
- Decay bias applied as one fused DVE op: S2 = (D * (-g_p)) + S where D is a
  host-precomputed |t-s| pattern (3 distinct patterns) with the diagonal
  entries set to 1e4 (folds the eye-mask: exp(-1e4*g) = 0 = exp(-100)/sigma).
- No-max softmax: logits bounded (~15), so exp without max subtraction is
  safe in f32; sigma accumulated by the Exp activation's accum_out.
- PV needs W[t,s]; W'[s,t] tiles are transposed via the DMA xbar engines
  (issued from the SP queue to keep the Act queue free).
- time_sig recovered from 8 augmented content rows (cos/sin) post-PV; the
  cos(s)*cc + sin(s)*ss pair-sum is folded into the proj matmul by
  duplicating the tsig columns of W_proj.
- b_eff folded into the proj matmul as an extra contraction row against a
  constant ones-row.
- proj partial computed on-core; host sums the two head-group partials.
  Residual x, b_proj, and W_proj@b_content folded in on the hg=0 core.

Perf notes (TimelineSim, 87.9us/core vs 104.1us baseline, HW-verified):
- freq/decay projections merged into one 72-row matmul group at 32-aligned
  partition bases (fq0@0:8, fq1@32:40, qd@64:72); partition bases moved back
  to 0 via DVE stream_shuffle (matmul tile_position offsets fault on HW).
- sigmoid via 0.5+0.5*tanh(x/2): tanh shares the Exp activation table so the
  Act engine never reloads tables; the affine part folds into the gneg
  evacuation scalars.
- PV outputs (om/oa) accumulate across 4-block groups in PSUM and evacuate
  as single [128,512]/[8,512] copies.
- software-pipelined schedule: exp(i) leads each iteration, S-matmuls(i+1)
  next, PV lagged 1 iteration; out-proj slices deferred to the task-starved
  late iterations with their PSUM evacuations one iteration behind the
  matmuls (nothing ever head-blocks an in-order queue); chunk-2/3
  projection tasks interleaved at their dependency deadlines; startup DMAs
  interleaved per k-tile so the first matmul starts ~2.5us in.
- DMA transposes issued from the SP queue; output DMAs deferred one
  iteration so the SP queue never holds long waits.
- evacuations balanced across engines: K-projection evac via DVE
  tensor_scalar (bias add), Q/CT/fq on Act, out-proj copies alternate.
"""
import math
import sys

sys.path.insert(0, "/opt/trn_rl_repo")

import ml_dtypes
import numpy as np

HEADS, NF, ND = 4, 4, 4
B, C, T = 4, 512, 2048
NBLK, WIN = 16, 384
DIAG_BIG = 1.0e4
BF16 = ml_dtypes.bfloat16

_CACHE = {}
import os as _os
os = _os
_TRANSPOSE_ON_SP = _os.environ.get("KM_T_SP", "1") == "1"
_BASE0 = _os.environ.get("KM_BASE0", "1") == "1"
_OA32 = _os.environ.get("KM_OA32", "0") == "1"


def _w0_of_block(i):
    return 128 * min(max(i - 1, 0), 13)


def _TL(pool, shape, dtype, tag):
    return pool.tile(shape, dtype, name=tag, tag=tag)


def _build_nc():
    import concourse.mybir as mybir
    import concourse.tile as tile
    from concourse import bacc

    dt = mybir.dt
    f32, bf16 = dt.float32, dt.bfloat16
    Alu = mybir.AluOpType
    Act = mybir.ActivationFunctionType

    nc = bacc.Bacc("TRN2", target_bir_lowering=False, debug=False, num_devices=8)

    # ---- DRAM I/O (per-core shards, host-prepared) ----
    xb_d = nc.dram_tensor("xb", [C, T], bf16, kind="ExternalInput")
    wqkc_d = nc.dram_tensor("wqkc", [C, 840], bf16, kind="ExternalInput")
    smalls_d = nc.dram_tensor("smalls", [128, 16], f32, kind="ExternalInput")
    cs_d = nc.dram_tensor("cs", [8, T], bf16, kind="ExternalInput")
    cs40_d = nc.dram_tensor("cs40", [40, T], bf16, kind="ExternalInput")
    csT_d = nc.dram_tensor("csT", [T, 8], bf16, kind="ExternalInput")
    d5_d = nc.dram_tensor("d5", [128, 3, WIN], f32, kind="ExternalInput")
    wp12_d = nc.dram_tensor("wp12", [256, C], bf16, kind="ExternalInput")
    wp3_d = nc.dram_tensor("wp3d", [17, C], bf16, kind="ExternalInput")
    out_d = nc.dram_tensor("out", [C, T], f32, kind="ExternalOutput")

    with tile.TileContext(nc) as tc:
        sing = tc.alloc_tile_pool(name="sing", bufs=1)
        work = tc.alloc_tile_pool(name="work", bufs=6)
        outp = tc.alloc_tile_pool(name="outp", bufs=8)
        n_s_bufs = 3 if (_OA32 or not _BASE0) else 2
        ps_pj = tc.alloc_tile_pool(name="ps_pj", bufs=2, space="PSUM")
        ps_s = tc.alloc_tile_pool(name="ps_s", bufs=n_s_bufs, space="PSUM")
        ps_om = tc.alloc_tile_pool(name="ps_om", bufs=2, space="PSUM")
        ps_oa = tc.alloc_tile_pool(name="ps_oa", bufs=1, space="PSUM")

        # ---- persistent SBUF (merged multi-k tiles: one DMA each) ----
        xb_sb = _TL(sing, [128, 4, T], bf16, tag="xb")
        wqkc_sb = _TL(sing, [128, 4, 840], bf16, tag="wqkc")
        smalls = _TL(sing, [128, 16], f32, tag="smalls")
        cs_sb = _TL(sing, [8, T], bf16, tag="cs")
        cs40 = _TL(sing, [40, T], bf16, tag="cs40")
        d5_sb = _TL(sing, [128, 3, WIN], f32, tag="d5")
        wp12_sb = _TL(sing, [128, 2, C], bf16, tag="wp12")
        wp3x0 = _TL(sing, [9, C], bf16, tag="wp3x0")
        wp3x1 = _TL(sing, [40, C], bf16, tag="wp3x1")
        CT_sb = _TL(sing, [128, NBLK, 264], bf16, tag="CT")

        # issue order = need order: weights+biases, x cols 0/1, small
        # tensors, proj weights, x cols 2/3
        xb_v = xb_d.ap().rearrange("(k p) t -> p k t", p=128)
        # interleave weight/x pieces per k-tile so the first accumulation
        # group can start as soon as its first operands land
        for k in range(4):
            nc.sync.dma_start(out=wqkc_sb[:, k, :],
                              in_=wqkc_d[128 * k:128 * (k + 1), :])
            nc.sync.dma_start(out=xb_sb[:, k, 0:512],
                              in_=xb_v[:, k, 0:512])
        nc.sync.dma_start(out=smalls, in_=smalls_d[:, :])
        nc.sync.dma_start(out=xb_sb[:, :, 512:1024], in_=xb_v[:, :, 512:1024])
        nc.sync.dma_start(out=cs_sb, in_=cs_d[:, :])
        nc.sync.dma_start(out=cs40, in_=cs40_d[:, :])
        nc.sync.dma_start(out=d5_sb, in_=d5_d[:, :, :])
        nc.sync.dma_start(out=wp12_sb,
                          in_=wp12_d.ap().rearrange("(k p) c -> p k c", p=128))
        nc.sync.dma_start(out=wp3x0, in_=wp3_d[0:9, :])
        if _BASE0 and not _OA32:
            nc.sync.dma_start(out=wp3x1[0:8, :], in_=wp3_d[9:17, :])
        else:
            nc.sync.dma_start(out=wp3x1[32:40, :], in_=wp3_d[9:17, :])
        nc.sync.dma_start(
            out=CT_sb[:, :, 256:264],
            in_=csT_d.ap().rearrange("(tt p) c -> p tt c", p=128),
        )
        for n in range(2, 4):
            nc.sync.dma_start(out=xb_sb[:, :, 512 * n:512 * (n + 1)],
                              in_=xb_v[:, :, 512 * n:512 * (n + 1)])

        xb = [xb_sb[:, k, :] for k in range(4)]
        wqT = [wqkc_sb[:, k, 0:256] for k in range(4)]
        wkT = [wqkc_sb[:, k, 256:512] for k in range(4)]
        wcT = [wqkc_sb[:, k, 512:768] for k in range(4)]
        wfdT = [wqkc_sb[:, k, 768:840] for k in range(4)]
        bq_sb = smalls[:, 0:2]
        bk_sb = smalls[:, 2:4]
        bfq_sb = smalls[0:40, 4:5]
        bqd_sb = smalls[64:72, 5:6]
        gco_sb = smalls[64:72, 6:8]

        # ---- working SBUF ----
        Q_sb = [_TL(sing, [128, T], bf16, tag=f"Q{h}") for h in range(2)]
        K_sb = [_TL(sing, [128, T], bf16, tag=f"K{h}") for h in range(2)]
        fq40 = _TL(sing, [64, T], bf16, tag="fq40")
        sig72 = _TL(sing, [96, T], f32, tag="sig72")
        gneg_sb = _TL(sing, [128, 2 * NBLK], f32, tag="gneg")
        Qaux0 = _TL(sing, [8, T], bf16, tag="Qaux0")
        Qaux1 = _TL(sing, [40, T], bf16, tag="Qaux1")
        Res = [_TL(sing, [128, T], bf16, tag=f"Res{h}") for h in range(2)]
        Ts40 = _TL(sing, [40, T], bf16, tag="Ts40")
        prod40 = _TL(sing, [40, T], bf16, tag="prod40")
        WnT = _TL(sing, [128, NBLK, 2, WIN], bf16, tag="WnT")
        if _BASE0:
            sig8 = _TL(sing, [32, T], f32, tag="sig8")
            fqh1 = _TL(sing, [32, T], bf16, tag="fqh1")
            Qaux1b = _TL(sing, [8, T], bf16, tag="Qaux1b")
            Ts8h1 = _TL(sing, [8, T], bf16, tag="Ts8h1")
            prod8h1 = _TL(sing, [8, T], bf16, tag="prod8h1")
        pat_of = lambda i: {0: 0, 15: 2}.get(i, 1)

        # constant ones row for the bias fold: memset rows 0:32 (partition
        # bases must be 32-aligned); rows 0:8 are overwritten by the prod
        # muls each group, row 8 stays 1.0 and folds b_eff into the matmul
        nc.gpsimd.memset(prod40[0:32, :], 1.0)
        if os.environ.get("KM_WARM_ACT", "0") == "1":
            # dummy act right at program start: hoists the 1.3us activation
            # table load into the input-DMA shadow
            warm = _TL(work, [1, 1], f32, tag="warm")
            nc.gpsimd.memset(warm, 0.0)
            warm2 = _TL(work, [1, 1], f32, tag="warm2")
            nc.scalar.activation(out=warm2, in_=warm, func=Act.Exp)
        if _BASE0:
            nc.gpsimd.memset(fq40[32:64, :], 0.0)
            nc.gpsimd.memset(sig72[64:96, :], 0.0)

        def emit_chunk_tasks(n):
            """Mini-tasks for projection chunk n (emitted interleaved)."""
            cols = slice(512 * n, 512 * (n + 1))

            def qk_task(h, wT, dst, bias, on_dve=False):
                def f():
                    p = _TL(ps_pj, [128, 512], f32, tag="proj")
                    for k in range(4):
                        nc.tensor.matmul(p, wT[k][:, 128 * h:128 * (h + 1)],
                                         xb[k][:, cols], start=(k == 0),
                                         stop=(k == 3))
                    if on_dve and os.environ.get("KM_K_DVE", "1") == "1":
                        nc.vector.tensor_scalar(out=dst[h][:, cols], in0=p,
                                                scalar1=bias[:, h:h + 1],
                                                scalar2=None, op0=Alu.add)
                    else:
                        nc.scalar.activation(out=dst[h][:, cols], in_=p,
                                             func=Act.Identity,
                                             bias=bias[:, h:h + 1])
                return f

            def fqd_task():
                # merged freq/decay projections: [72, 512] psum
                pf = _TL(ps_pj, [72, 512], f32, tag="proj")
                for k in range(4):
                    nc.tensor.matmul(pf, wfdT[k], xb[k][:, cols],
                                     start=(k == 0), stop=(k == 3))
                if os.environ.get("KM_FQ_DVE", "0") == "1":
                    nc.vector.tensor_scalar(out=fq40[0:40, cols],
                                            in0=pf[0:40, :], scalar1=bfq_sb,
                                            scalar2=None, op0=Alu.add)
                else:
                    nc.scalar.activation(out=fq40[0:40, cols],
                                         in_=pf[0:40, :],
                                         func=Act.Identity, bias=bfq_sb)
                # sigmoid(x) = 0.5 + 0.5*tanh(x/2); tanh shares the Exp
                # act table (no table reloads). The affine part is folded
                # into the gneg evacuation below.
                nc.scalar.activation(out=sig72[64:72, cols], in_=pf[64:72, :],
                                     func=Act.Tanh, scale=0.5, bias=bqd_sb)
                # emit the Pool muls right after their producer acts so the
                # in-order Pool queue never holds long waits
                nc.gpsimd.tensor_mul(Qaux0[:, cols], cs_sb[:, cols],
                                     fq40[0:8, cols])
                if _BASE0:
                    ident = list(range(32))
                    nc.vector.stream_shuffle(out=fqh1[0:32, cols],
                                             in_=fq40[32:64, cols], mask=ident)
                    nc.vector.stream_shuffle(out=sig8[0:32, cols],
                                             in_=sig72[64:96, cols], mask=ident)
                    nc.gpsimd.tensor_mul(Qaux1b[:, cols], cs_sb[:, cols],
                                         fqh1[0:8, cols])
                else:
                    nc.gpsimd.tensor_mul(Qaux1[32:40, cols], cs40[32:40, cols],
                                         fq40[32:40, cols])

            def ct_task(m):
                def f():
                    pc = _TL(ps_pj, [128, 512], f32, tag="proj")
                    for j in range(2):
                        rows = slice(128 * (2 * m + j), 128 * (2 * m + j + 1))
                        for k in range(4):
                            nc.tensor.matmul(pc[:, 256 * j:256 * (j + 1)],
                                             xb[k][:, rows], wcT[k],
                                             start=(k == 0), stop=(k == 3))
                    if os.environ.get("KM_CT_DVE", "0") == "1":
                        nc.vector.tensor_copy(
                            out=CT_sb[:, 2 * m:2 * m + 2, 0:256], in_=pc)
                    else:
                        nc.scalar.activation(
                            out=CT_sb[:, 2 * m:2 * m + 2, 0:256],
                            in_=pc, func=Act.Copy)
                return f

            def gneg_task():
                # gneg = sum_d gco_d * sigmoid_d = 0.5*sum(gco)*tanh + (-1.25)
                pg = _TL(ps_pj, [128, 8], f32, tag="proj")
                for j in range(4):
                    i = 4 * n + j
                    if _BASE0:
                        nc.tensor.matmul(pg[:, 2 * j:2 * j + 2],
                                         sig8[0:8, 128 * i:128 * (i + 1)],
                                         smalls[0:8, 6:8],
                                         start=True, stop=True)
                    else:
                        nc.tensor.matmul(pg[:, 2 * j:2 * j + 2],
                                         sig72[64:72, 128 * i:128 * (i + 1)],
                                         gco_sb, start=True, stop=True)
                nc.vector.tensor_scalar(out=gneg_sb[:, 8 * n:8 * n + 8], in0=pg,
                                        scalar1=0.5, scalar2=-1.25,
                                        op0=Alu.mult, op1=Alu.add)

            return [qk_task(0, wqT, Q_sb, bq_sb),
                    qk_task(0, wkT, K_sb, bk_sb, on_dve=True),
                    qk_task(1, wqT, Q_sb, bq_sb),
                    qk_task(1, wkT, K_sb, bk_sb, on_dve=True),
                    fqd_task, ct_task(2 * n), ct_task(2 * n + 1), gneg_task]

        blk_state = {}

        def emit_A_mm(i):
            # S matmuls for block i (both heads)
            s0, w0 = 128 * i, _w0_of_block(i)
            s2s = []
            for h in range(2):
                sp = _TL(ps_s, [128, WIN], f32, tag="S")
                nc.tensor.matmul(sp, Q_sb[h][:, s0:s0 + 128],
                                 K_sb[h][:, w0:w0 + WIN], start=True, stop=False)
                if h == 0:
                    nc.tensor.matmul(sp, Qaux0[:, s0:s0 + 128],
                                     cs_sb[:, w0:w0 + WIN], start=False, stop=True)
                elif _BASE0:
                    nc.tensor.matmul(sp, Qaux1b[:, s0:s0 + 128],
                                     cs_sb[:, w0:w0 + WIN], start=False, stop=True)
                else:
                    nc.tensor.matmul(sp, Qaux1[32:40, s0:s0 + 128],
                                     cs40[32:40, w0:w0 + WIN], start=False, stop=True)
                s2s.append(sp)
            blk_state[i] = s2s

        def emit_A_stt(i):
            # decay bias for block i (in-place on the S psum tiles)
            for h, sp in enumerate(blk_state[i]):
                nc.vector.scalar_tensor_tensor(
                    out=sp, in0=d5_sb[:, pat_of(i), :],
                    scalar=gneg_sb[:, 2 * i + h:2 * i + h + 1],
                    in1=sp, op0=Alu.mult, op1=Alu.add)

        def emit_exp(i):
            # softmax exponent for block i (frees the S psum tiles asap;
            # emitted at iteration start so it leads the Act queue)
            s2s = blk_state.pop(i)
            ws = []
            for h in range(2):
                wexp = _TL(work, [128, WIN], bf16, tag="Wexp")
                sigma = _TL(work, [128, 1], f32, tag="sigma")
                nc.scalar.activation(out=wexp, in_=s2s[h], func=Act.Exp,
                                     accum_out=sigma)
                ws.append((wexp, sigma))
            blk_state[(i, "e")] = ws

        def emit_norm(i):
            # normalize + transpose for block i
            ws = blk_state.pop((i, "e"))
            wn2 = _TL(work, [128, 2 * WIN], bf16, tag="Wn2")
            for h in range(2):
                wexp, sigma = ws[h]
                recip = _TL(work, [128, 1], f32, tag="recip")
                nc.vector.reciprocal(out=recip, in_=sigma)
                nc.vector.tensor_scalar_mul(wn2[:, WIN * h:WIN * (h + 1)],
                                            wexp, recip)
            teng = nc.sync if _TRANSPOSE_ON_SP else nc.scalar
            teng.dma_start_transpose(
                out=WnT[:, i, :, :].rearrange("p hh (k e) -> p (hh k) e", e=128),
                in_=wn2)

        def emit_pv(i, oms, oa):
            # PV for block i accumulating into the 4-block group tiles
            b = i % 4
            base = min(max(i - 1, 0), 13)
            for h in range(2):
                for j in range(3):
                    tt = base + j
                    rhs = WnT[:, i, h, 128 * j:128 * (j + 1)]
                    nc.tensor.matmul(oms[h][:, 128 * b:128 * (b + 1)],
                                     CT_sb[:, tt, 128 * h:128 * (h + 1)],
                                     rhs, start=(j == 0), stop=(j == 2))
                    if _BASE0 and not _OA32:
                        oa_slice = oa[h][:, 128 * b:128 * (b + 1)]
                    else:
                        oa_slice = oa[32 * h:32 * h + 8, 128 * b:128 * (b + 1)]
                    nc.tensor.matmul(oa_slice, CT_sb[:, tt, 256:264],
                                     rhs, start=(j == 0), stop=(j == 2))

        def emit_tail_evac(g, oms, oa):
            # evacuate group-g PV results + compute prod rows
            cols = slice(512 * g, 512 * (g + 1))
            for h in range(2):
                nc.vector.tensor_copy(out=Res[h][:, cols], in_=oms[h])
            if _BASE0 and not _OA32:
                if os.environ.get("KM_TS_DVE", "0") == "1":
                    nc.vector.tensor_copy(out=Ts40[0:8, cols], in_=oa[0])
                    nc.vector.tensor_copy(out=Ts8h1[:, cols], in_=oa[1])
                else:
                    nc.scalar.activation(out=Ts40[0:8, cols], in_=oa[0],
                                         func=Act.Copy)
                    nc.scalar.activation(out=Ts8h1[:, cols], in_=oa[1],
                                         func=Act.Copy)
                nc.gpsimd.tensor_mul(prod40[0:8, cols], cs_sb[:, cols],
                                     Ts40[0:8, cols])
                nc.gpsimd.tensor_mul(prod8h1[:, cols], cs_sb[:, cols],
                                     Ts8h1[:, cols])
            else:
                nc.scalar.activation(out=Ts40[0:8, cols], in_=oa[0:8, :],
                                     func=Act.Copy)
                nc.scalar.activation(out=Ts40[32:40, cols], in_=oa[32:40, :],
                                     func=Act.Copy)
                nc.gpsimd.tensor_mul(prod40[0:8, cols], cs_sb[:, cols],
                                     Ts40[0:8, cols])
                nc.gpsimd.tensor_mul(prod40[32:40, cols], cs40[32:40, cols],
                                     Ts40[32:40, cols])

        def emit_tail_ot_mm(g, ot):
            # matmul half of one out-projection slice; the evacuation runs
            # one iteration later so it never head-blocks a queue
            cols = slice(512 * g, 512 * (g + 1))
            pp = _TL(ps_pj, [128, 512], f32, tag="proj")
            osl = slice(128 * ot, 128 * (ot + 1))
            nc.tensor.matmul(pp, wp12_sb[:, 0, osl], Res[0][:, cols],
                             start=True, stop=False)
            nc.tensor.matmul(pp, wp12_sb[:, 1, osl], Res[1][:, cols],
                             start=False, stop=False)
            nc.tensor.matmul(pp, wp3x0[:, osl], prod40[0:9, cols],
                             start=False, stop=False)
            if _BASE0 and not _OA32:
                nc.tensor.matmul(pp, wp3x1[0:8, osl], prod8h1[:, cols],
                                 start=False, stop=True)
            else:
                nc.tensor.matmul(pp, wp3x1[32:40, osl], prod40[32:40, cols],
                                 start=False, stop=True)
            return pp

        def emit_tail_ot_ob(g, ot, pp):
            cols = slice(512 * g, 512 * (g + 1))
            osl = slice(128 * ot, 128 * (ot + 1))
            ob = _TL(outp, [128, 512], f32, tag="ob")
            obm = os.environ.get("KM_OB", "mix")
            if obm == "dve" or (obm == "mix" and ot % 2 == 0):
                nc.vector.tensor_copy(out=ob, in_=pp)
            else:
                nc.scalar.activation(out=ob, in_=pp, func=Act.Copy)
            pending_dmas.append(
                lambda osl=osl, cols=cols, ob=ob:
                nc.sync.dma_start(out=out_d[osl, cols], in_=ob))

        # ---- schedule: software-pipelined blocks with interleaved proj
        # mini-tasks and deferred output-DMA issue ----
        group_tiles = {}

        def get_group(g):
            if g not in group_tiles:
                oms = [_TL(ps_om, [128, 512], f32, tag="om") for _ in range(2)]
                if _BASE0 and not _OA32:
                    oa = [_TL(ps_oa, [8, 512], f32, tag=f"oa{h}")
                          for h in range(2)]
                else:
                    oa = _TL(ps_oa, [40, 512], f32, tag="oa")
                group_tiles[g] = (oms, oa)
            return group_tiles[g]

        for task in emit_chunk_tasks(0):
            task()
        for task in emit_chunk_tasks(1):
            task()
        # chunk-2/3 tasks placed as late as their consumers allow (PV runs
        # one iteration behind S/softmax, so deadlines are generous)
        PV_LAG = int(os.environ.get("KM_PV_LAG", "1"))
        c2, c3 = emit_chunk_tasks(2), emit_chunk_tasks(3)
        sched = {2: [c2[0]], 3: [c2[1], c2[2]], 4: [c2[3]], 5: [c2[4]],
                 6: [c2[7], c2[5]], 7: [c3[0], c2[6]], 8: [c3[1], c3[2]],
                 9: [c3[3], c3[4]], 10: [c3[7]], 11: [c3[5]], 13: [c3[6]]}
        pending_dmas = []

        EVAC_D = int(os.environ.get("KM_EVAC_D", "2"))
        OT_MIN = EVAC_D + 1
        OT_D = max(int(os.environ.get("KM_OT_D", "3")), OT_MIN)
        OT_STRIDE = int(os.environ.get("KM_OT_STRIDE", "1"))

        def do_pv(j, cur_i):
            oms, oa = get_group(j // 4)
            emit_pv(j, oms, oa)
            if j % 4 == 3:
                g = j // 4
                tiles = group_tiles.pop(g)
                sched.setdefault(cur_i + EVAC_D, []).append(
                    lambda g=g, tiles=tiles: emit_tail_evac(g, *tiles))
                late = os.environ.get("KM_OT_LATE", "1") == "1"
                for ot in range(4):
                    tgt = cur_i + OT_D + ot * OT_STRIDE
                    if late:
                        tgt = max(cur_i + EVAC_D + 1 + ot,
                                  int(os.environ.get("KM_LS", "10")) + g + ot)

                    def mm_then_ob(g=g, ot=ot, tgt=tgt):
                        pp = emit_tail_ot_mm(g, ot)
                        sched.setdefault(tgt + 1, []).insert(
                            0, lambda: emit_tail_ot_ob(g, ot, pp))
                    sched.setdefault(tgt, []).append(mm_then_ob)

        emit_A_mm(0)
        emit_A_stt(0)
        for i in range(NBLK):
            flush, pending_dmas = pending_dmas, []
            for d in flush:
                d()
            emit_exp(i)
            if i + 1 < NBLK:
                emit_A_mm(i + 1)
                emit_A_stt(i + 1)
            emit_norm(i)
            for task in sched.pop(i, ()):
                task()
            if i >= PV_LAG:
                do_pv(i - PV_LAG, i)
        for j in range(NBLK - PV_LAG, NBLK):
            do_pv(j, NBLK + (j - (NBLK - PV_LAG)))
        i = NBLK
        while sched and i < NBLK + 32:
            for task in sched.pop(i, ()):
                task()
            i += 1
        for d in pending_dmas:
            d()

        for pool in (ps_oa, ps_om, ps_s, ps_pj, outp, work, sing):
            pool.release()

    nc.compile()
    return nc


def _cos_sin():
    t = np.arange(T, dtype=np.float64)
    per = np.arange(1, NF + 1, dtype=np.float64)
    ang = 2 * math.pi * t[None, :] / per[:, None]
    return np.cos(ang).astype(np.float32), np.sin(ang).astype(np.float32)


def _d_patterns():
    d5 = np.empty((128, 3, WIN), np.float32)
    p = np.arange(128)[:, None]
    j = np.arange(WIN)[None, :]
    for k, off in enumerate((0, 128, 256)):
        d = np.abs(p + off - j).astype(np.float32)
        d[p + off == j] = DIAG_BIG
        d5[:, k, :] = d
    return d5


_COS, _SIN = _cos_sin()
_D5 = _d_patterns()


def _prep_core_inputs(inputs, b, hg):
    f32 = np.float32
    x_b = np.ascontiguousarray(np.asarray(inputs["x"])[b], dtype=f32)
    hsl = slice(hg * 256, (hg + 1) * 256)
    fsl = slice(hg * 8, (hg + 1) * 8)
    s = f32(1.0 / math.sqrt(128.0))
    cosT, sinT = _COS, _SIN
    cs = np.concatenate([cosT, sinT], 0)  # [8, T]

    Wq = np.asarray(inputs["W_query"], f32)[hsl] * s
    bq = np.asarray(inputs["b_query"], f32)[hsl] * s
    Wk = np.asarray(inputs["W_key"], f32)[hsl]
    bk = np.asarray(inputs["b_key"], f32)[hsl]
    Wc = np.asarray(inputs["W_content"], f32)[hsl]
    Wf = np.asarray(inputs["W_qfreq"], f32)[fsl] * f32(0.5)
    bf = np.asarray(inputs["b_qfreq"], f32)[fsl] * f32(0.5)
    Wd = np.asarray(inputs["W_qdecay"], f32)[fsl]
    bd = np.asarray(inputs["b_qdecay"], f32)[fsl]
    Wp = np.asarray(inputs["W_proj"], f32)
    Wp_hg = Wp[:, hg * 264:(hg + 1) * 264]

    # wfd72 rows: fq_h0 x2 @0:8, fq_h1 x2 @32:40, qd_h0 @64:68, qd_h1 @68:72
    wfd72 = np.zeros((72, C), f32)
    wfd72[0:4] = Wf[0:4]
    wfd72[4:8] = Wf[0:4]
    wfd72[32:36] = Wf[4:8]
    wfd72[36:40] = Wf[4:8]
    wfd72[64:68] = Wd[0:4]
    wfd72[68:72] = Wd[4:8]

    gco = np.zeros((8, 2), f32)
    dvec = -(np.arange(1, ND + 1, dtype=f32) / 4)
    gco[0:4, 0] = dvec
    gco[4:8, 1] = dvec

    wp12 = np.concatenate([Wp_hg[:, 0:128].T, Wp_hg[:, 132:260].T], axis=0)

    if hg == 0:
        b_eff = np.asarray(inputs["b_proj"], f32).copy()
        bc = np.asarray(inputs["b_content"], f32)
        for h in range(HEADS):
            b_eff += Wp[:, 132 * h:132 * h + 128] @ bc[128 * h:128 * h + 128]
    else:
        b_eff = np.zeros(C, f32)

    # wp3d rows 0:8 = tsig cols h0 (cos dup, sin dup), row 8 = b_eff,
    # rows 9:17 = tsig cols h1
    wp3d = np.zeros((17, C), f32)
    wp3d[0:4] = Wp_hg[:, 128:132].T
    wp3d[4:8] = Wp_hg[:, 128:132].T
    wp3d[8] = b_eff
    wp3d[9:13] = Wp_hg[:, 260:264].T
    wp3d[13:17] = Wp_hg[:, 260:264].T

    wqkc = np.concatenate([Wq.T, Wk.T, Wc.T, wfd72.T], axis=1)  # [512, 840]
    smalls = np.zeros((128, 16), f32)
    smalls[:, 0:2] = bq.reshape(2, 128).T
    smalls[:, 2:4] = bk.reshape(2, 128).T
    smalls[0:8, 4] = np.concatenate([bf[0:4], bf[0:4]])
    smalls[32:40, 4] = np.concatenate([bf[4:8], bf[4:8]])
    smalls[64:72, 5] = np.concatenate([bd[0:4], bd[4:8]]) * f32(0.5)
    smalls[64:72, 6:8] = gco
    smalls[0:8, 6:8] = gco

    cs40 = np.zeros((40, T), f32)
    cs40[0:8] = cs
    cs40[32:40] = cs
    return {
        "xb": x_b.astype(BF16),
        "wqkc": np.ascontiguousarray(wqkc).astype(BF16),
        "smalls": smalls,
        "cs": cs.astype(BF16),
        "cs40": cs40.astype(BF16),
        "csT": np.ascontiguousarray(cs.T).astype(BF16),
        "d5": _D5,
        "wp12": np.ascontiguousarray(wp12).astype(BF16),
        "wp3d": np.ascontiguousarray(wp3d).astype(BF16),
    }


def get_nc():
    if "nc" not in _CACHE:
        _CACHE["nc"] = _build_nc()
    return _CACHE["nc"]


def make_in_maps(inputs):
    return [_prep_core_inputs(inputs, c // 2, c % 2) for c in range(8)]


def kernel(**inputs):
    from concourse.bass_utils import run_bass_kernel_spmd

    nc = get_nc()
    in_maps = make_in_maps(inputs)
    res = run_bass_kernel_spmd(nc, in_maps, core_ids=list(range(8)))
    x = np.asarray(inputs["x"], np.float32)
    out = np.empty((B, C, T), np.float32)
    for b in range(B):
        np.add(res.results[2 * b]["out"], res.results[2 * b + 1]["out"], out=out[b])
        out[b] += x[b]
    return out
